# revision 10
# baseline (speedup 1.0000x reference)
"""Trainium2 Bass kernel for nn_ComparisonLayer (v4).

Computes, for x:(L,B,D) with L=512,B=2,D=256,C=128,O=64:
    xb  = layernorm(transpose(x,(1,0,2)))          # (B,L,D)
    a   = xb@w1+b1 ; b = xb@w2+b2                  # (B,L,C)
    out[b,l,m,o] = sum_c a[b,l,c]*b[b,m,c]*w3[c,o] + b3[o]
                   + (a@w4)[b,l,o] - (b@w4)[b,m,o] # (B,L,L,O)

Sharding: 8 cores, core k handles batch k//4 and l-block q=k%4.

Structure per core (all row terms folded into one K=2 rank matmul):
    out[l, o, m] = (aT * w3[:,o]).T @ bT           # main MM, K=128 N=512
                 + [ones; a4T[o]].T @ [negb4T[o]; ones]   # K=2 rank MM
Drains are pure PSUM->SBUF copies of 2-o (2-bank) packs, split ACT/DVE
5:3.  b4T comes directly from xn via host-folded w24 = w2g @ w4.  LN
stats use bn_stats/bn_aggr (one ACT table set, less ACT work).  The
ones planes for the rank matmuls and the warm-up operand are DMA'd
from DRAM.  One xall DMA (split DMAs block the in-order engine queues
on arrival sems).  Aggressive PE warm-up: the HAM clock gate needs
~3.4us of sustained matmul activity before the PE runs at 2.4 GHz.
Device output layout (l, (o, m_dev)) bf16; host un-rotates m (tiles
are rotated so slot 0 is the core's own l-block) and transposes to
(l, m, o) fp32.
"""

import numpy as np
import ml_dtypes

import concourse.bacc as bacc
import concourse.bass as bass
import concourse.mybir as mybir
import concourse.tile as tile
from concourse.bass_utils import run_bass_kernel_spmd

L, B, D, C, O = 512, 2, 256, 128, 64
NCORES = 8
LBLK = 128                   # l rows per core
NT = 4                       # row tiles of 128

F32 = mybir.dt.float32
BF16 = mybir.dt.bfloat16

# wbfa (bf16) columns: identity + w1 halves
WA_ID = 0            # [0:128)
WA_W1 = 128          # [128:384)  w1g halves (h p) c -> p (h c)
WA_N = 384
# wbfb (bf16) columns: w2 halves + w24 halves + w4
WB_W2 = 0            # [0:256)
WB_W24 = 256         # [256:384)  w24 halves (h p) o -> p (h o)
WB_W4 = 384          # [384:448)  w4 (C, O)
WB_N = 448
# wf32 columns
WF_W3 = 0            # [0:64)   w3 (C, O)
WF_B1 = 64           # b1e
WF_B2 = 65           # b2e
WF_NB = 66           # nbias = b3 - b2e@w4, partitions 0..63
WF_N = 67
# ones planes (DRAM): [4, 8192] rowRep rows 32r+1 + [4, 2048] rk rows 32r
ONES_N = 16 * L + 16 * 128   # 10240

# output blocks (o ranges per DMA)
BOUNDS = [0, 2, 4, 12, 20, 28, 36, 44, 52, 58, 62, 64]
OBLK_MAX = max(b - a for a, b in zip(BOUNDS, BOUNDS[1:]))


def _build():
    nc = bacc.Bacc("TRN2", target_bir_lowering=False, debug=False)

    xall_d = nc.dram_tensor("xall", (128, NT, D), BF16, kind="ExternalInput")
    wbfa_d = nc.dram_tensor("wbfa", (128, WA_N), BF16, kind="ExternalInput")
    wbfb_d = nc.dram_tensor("wbfb", (128, WB_N), BF16, kind="ExternalInput")
    wf32_d = nc.dram_tensor("wf32", (128, WF_N), F32, kind="ExternalInput")
    ones_d = nc.dram_tensor("ones", (4, ONES_N), BF16, kind="ExternalInput")
    out_d = nc.dram_tensor("out", (LBLK, O * L), BF16, kind="ExternalOutput")

    AX = mybir.AxisListType.X
    ALU = mybir.AluOpType
    ACT = mybir.ActivationFunctionType

    with tile.TileContext(nc) as tc:
        with (
            tc.tile_pool(name="const", bufs=1) as cp,
            tc.tile_pool(name="work", bufs=2) as wp,
            tc.tile_pool(name="aw", bufs=3) as awp,
            tc.tile_pool(name="ob", bufs=4) as obp,
            tc.tile_pool(name="ps", bufs=3, space="PSUM") as pm,
            tc.tile_pool(name="tp", bufs=1, space="PSUM") as tpm,
            tc.tile_pool(name="wf", bufs=1, space="PSUM") as wfm,
        ):
            # ---------- input DMAs (sync: x + ones planes; scalar: weights)
            xall = cp.tile([128, NT, D], BF16)
            wbfa = cp.tile([128, WA_N], BF16)
            wbfb = cp.tile([128, WB_N], BF16)
            wf32 = cp.tile([128, WF_N], F32)
            rowRep = cp.tile([128, 16 * L], BF16)
            rk = cp.tile([128, 16 * 128], BF16)
            nc.sync.dma_start(xall[:], xall_d[:])
            nc.sync.dma_start(wbfa[:], wbfa_d[:])
            nc.sync.dma_start(wbfb[:], wbfb_d[:])
            nc.sync.dma_start(wf32[:], wf32_d[:])
            nc.sync.dma_start(rowRep[1:128:32, :], ones_d[:, 0:16 * L])
            nc.sync.dma_start(rk[0:128:32, :], ones_d[:, 16 * L:ONES_N])

            id128 = wbfa[:, WA_ID:WA_ID + 128]
            w4s = wbfb[:, WB_W4:WB_W4 + O]

            onesb = cp.tile([128, 512], BF16)
            nc.vector.memset(onesb[:], 1.0)
            # dummy 1-col Sqrt pulls the ACT table load off the LN chain
            tblw = wp.tile([128, 1], F32, tag="tblw")
            nc.vector.memset(tblw[:], 1.0)
            nc.scalar.activation(tblw[:], tblw[:], ACT.Sqrt)

            # long-lived PSUM accumulators (allocate first: pin ring slots)
            bps_t = pm.tile([128, 2 * L], F32, tag="ps", name="bps")
            bps = bps_t[:, 0:L]
            b4ps_t = pm.tile([128, 2 * L], F32, tag="ps", name="b4ps")
            b4ps = b4ps_t[0:O, 0:L]

            # ---------- HAM warm-up ----------
            # The PE clock gate (HAM) needs ~3.4us of sustained matmul
            # activity to open to 2.4 GHz, and one mostly-idle window
            # re-throttles it; transposes do not count as activity.  Keep a
            # filler matmul stream through every PE idle window, via a
            # dedicated PSUM bank so fillers never contend with real work.
            wfill = wfm.tile([128, L], F32, tag="wf", name="wfill")

            def warm(n, nm):
                for wi in range(n):
                    nc.tensor.matmul(wfill[:], onesb[:, 0:128],
                                     onesb[:, 0:L], start=True, stop=True)

            warm(7, "wburst")

            # ---------- per-tile LN + transpose + bT/b4 matmuls ----------
            stat6 = wp.tile([128, NT, 6], F32, tag="stat6")
            mv = wp.tile([128, NT, 2], F32, tag="mv")
            rstd = wp.tile([128, NT], F32, tag="rstd")
            xn = cp.tile([128, NT, D], BF16)
            xnT = cp.tile([128, 2, NT, 128], BF16)
            aT_c = cp.tile([C, 128], BF16)
            a4bf = cp.tile([O, 128], BF16)
            bT_c = cp.tile([C, L], BF16)
            nb = cp.tile([O, L], BF16)

            aw_tiles = {}

            def build_aw8(k):  # o's [8k, 8k+8); ring bufs pace the
                # builds with the loop (the scheduler otherwise hoists
                # them into one long DVE wall that starves the rank chain)
                t = awp.tile([C, 8, 128], BF16, tag="aw", name=f"aw{k}")
                a_bc = aT_c[:].unsqueeze(1).broadcast_to((C, 8, 128))
                w_bc = wf32[:, WF_W3 + 8 * k:WF_W3 + 8 * k + 8] \
                    .unsqueeze(2).broadcast_to((C, 8, 128))
                nc.vector.tensor_tensor(t[:], a_bc, w_bc, op=ALU.mult)
                aw_tiles[k] = t

            for s in range(NT):
                xs = xall[:, s, :]
                sc = slice(s, s + 1)
                nc.vector.bn_stats(stat6[:, s, :], xs)
                nc.vector.bn_aggr(mv[:, s, :], stat6[:, s, :])
                nc.scalar.activation(
                    rstd[:, sc], mv[:, s, 1:2], ACT.Sqrt)
                nc.vector.reciprocal(rstd[:, sc], rstd[:, sc])
                nc.vector.tensor_scalar(
                    xn[:, s, :], xs, mv[:, s, 0:1], rstd[:, sc],
                    op0=ALU.subtract, op1=ALU.mult)
                for h in range(2):
                    tp = tpm.tile([128, 512], BF16, tag="tp", name=f"tp{s}{h}")
                    nc.tensor.transpose(
                        tp[:, 0:128], xn[:, s, h * 128:(h + 1) * 128], id128)
                    if h == 0:
                        nc.scalar.copy(xnT[:, h, s, :], tp[:, 0:128])
                    else:
                        nc.vector.tensor_copy(xnT[:, h, s, :], tp[:, 0:128])
                # bT rows: accumulate w2g.T @ xnT into bps columns of tile s
                for h in range(2):
                    nc.tensor.matmul(
                        bps[:, s * 128:(s + 1) * 128],
                        wbfb[:, WB_W2 + h * 128:WB_W2 + (h + 1) * 128],
                        xnT[:, h, s, :],
                        start=(s == 0 and h == 0), stop=(s == 3 and h == 1),
                    )
                # b4T rows: w24.T @ xnT (direct, skips bT).  Per-tile
                # start/stop groups: start=True only clears has_written
                # bits, previously written column VALUES survive.
                for h in range(2):
                    nc.tensor.matmul(
                        b4ps[:, s * 128:(s + 1) * 128],
                        wbfb[:, WB_W24 + h * O:WB_W24 + (h + 1) * O],
                        xnT[:, h, s, :],
                        start=(h == 0), stop=(h == 1),
                    )
                # negb4T slice for this tile (gates rank matmuls)
                nc.scalar.activation(
                    nb[:, s * 128:(s + 1) * 128],
                    b4ps[:, s * 128:(s + 1) * 128], ACT.Identity,
                    bias=wf32[0:O, WF_NB:WF_NB + 1], scale=-1.0)
                if s == 0:
                    # aT / a4T chain (tile 0 == this core's own l-block)
                    aps_t = pm.tile([128, 2 * L], F32, tag="ps", name="aps")
                    for h in range(2):
                        nc.tensor.matmul(
                            aps_t[:, 0:128],
                            wbfa[:, WA_W1 + h * 128:WA_W1 + (h + 1) * 128],
                            xnT[:, h, 0, :], start=(h == 0), stop=(h == 1),
                        )
                    nc.vector.tensor_scalar_add(
                        aT_c[:], aps_t[:, 0:128], wf32[:, WF_B1:WF_B1 + 1])
                    a4ps_t = pm.tile([128, 2 * L], F32, tag="ps", name="a4ps")
                    nc.tensor.matmul(a4ps_t[0:O, 0:128], w4s, aT_c[:],
                                     start=True, stop=True)
                    nc.scalar.copy(a4bf[:], a4ps_t[0:O, 0:128])
                    # a4T strip replicas -> rk partitions 32r+1
                    for r in range(4):
                        eng = nc.sync if r % 2 == 0 else nc.scalar
                        eng.dma_start(
                            rk[32 * r + 1:32 * r + 2, :], a4bf[r:O:4, :])
                warm(2, f"wt{s}")

            # flatten negb4T strips into rowRep rows 32r; the g<4 columns
            # ship first (tiny transfers) so the first rank matmuls
            # unblock early, the rest follows under the running loop
            for r in range(4):
                eng = nc.sync if r % 2 == 0 else nc.scalar
                eng.dma_start(rowRep[32 * r:32 * r + 1, 0:4 * L],
                              nb[r:16:4, :])
            for r in range(4):
                eng = nc.sync if r % 2 == 0 else nc.scalar
                eng.dma_start(rowRep[32 * r:32 * r + 1, 4 * L:16 * L],
                              nb[16 + r:O:4, :])
            # bT_c (bias add) gates the main matmuls
            nc.vector.tensor_scalar_add(
                bT_c[:], bps[:], wf32[:, WF_B2:WF_B2 + 1])
            for k in range(2):
                build_aw8(k)
            # bridge the PE from the preamble into the main loop: the first
            # rank matmuls wait on the negb4T flatten chain
            warm(14, "wbridge")

            # ---------- main loop: 16 groups of 4 o's, 2-o PSUM packs ------
            blk_of = {}
            for bs, be in zip(BOUNDS, BOUNDS[1:]):
                for o in range(bs, be):
                    blk_of[o] = (bs, be)

            # p%3==1 -> DVE (11 packs), else ACT (21): balances ACT drains
            # against DVE drains + aw builds

            ob = None
            for g in range(O // 4):
                packA = pm.tile([128, 2 * L], F32, tag="ps", name=f"pkA{g}")
                packB = pm.tile([128, 2 * L], F32, tag="ps", name=f"pkB{g}")
                packs = (packA, packB)
                awb = aw_tiles[g // 2]
                # main matmuls first (start=True)
                for j in range(4):
                    o = 4 * g + j
                    dst = packs[j // 2][:, (j % 2) * L:(j % 2 + 1) * L]
                    nc.tensor.matmul(dst, awb[:, o % 8, :], bT_c[:],
                                     start=True, stop=False)
                # rank matmuls: [ones; a4T[o]] x [negb4T[o]; ones]
                for j in range(4):
                    dst = packs[j // 2][:, (j % 2) * L:(j % 2 + 1) * L]
                    nc.tensor.matmul(
                        dst,
                        rk[32 * j:32 * j + 2, g * 128:(g + 1) * 128],
                        rowRep[32 * j:32 * j + 2, g * L:(g + 1) * L],
                        start=False, stop=True, tile_position=(32 * j, 0),
                    )
                # drains: 2-o pure copies, ACT:DVE = 5:3
                for half in range(2):
                    p_idx = 2 * g + half
                    o0 = 2 * p_idx
                    bs, be = blk_of[o0]
                    if o0 == bs:
                        ob = obp.tile([128, OBLK_MAX * L], BF16, tag="ob",
                                      name=f"ob{p_idx}")
                    sl = ob[:, (o0 - bs) * L:(o0 - bs + 2) * L]
                    if p_idx % 3 == 1:
                        nc.vector.tensor_copy(sl, packs[half][:])
                    else:
                        nc.scalar.copy(sl, packs[half][:])
                    if o0 + 2 == be:
                        nc.sync.dma_start(
                            out_d[:, bs * L:be * L], ob[:, 0:(be - bs) * L])
                # build one aw block ahead (after drains: drain priority)
                if g % 2 == 1 and (g + 3) // 2 < O // 8:
                    build_aw8((g + 3) // 2)

    nc.compile()
    return nc


_CACHE = {}


def _get_nc():
    if "nc" not in _CACHE:
        _CACHE["nc"] = _build()
    return _CACHE["nc"]


def _make_in_maps(x, ln_gamma, ln_beta, w1, b1, w2, b2, w3, b3, w4):
    x = np.ascontiguousarray(x, dtype=np.float32)
    g = np.asarray(ln_gamma, np.float32)
    be = np.asarray(ln_beta, np.float32)
    w1 = np.asarray(w1, np.float32)
    w2 = np.asarray(w2, np.float32)
    # fold the LN affine into the first-layer weights:
    # (xn*g + be) @ w = xn @ (g[:,None]*w) + be @ w
    w1g = g[:, None] * w1
    w2g = g[:, None] * w2
    b1e = np.asarray(b1, np.float32) + be @ w1
    b2e = np.asarray(b2, np.float32) + be @ w2
    w3c = np.asarray(w3, np.float32)
    w4f = np.asarray(w4, np.float32)
    b3f = np.asarray(b3, np.float32)
    w24 = w2g @ w4f                      # (D, O)
    nbias = b3f - b2e @ w4f              # (O,)

    bf = ml_dtypes.bfloat16
    wbfa = np.zeros((128, WA_N), dtype=bf)
    wbfa[:, WA_ID:WA_ID + 128] = np.eye(128, dtype=np.float32).astype(bf)
    wbfa[:, WA_W1:WA_W1 + 256] = \
        w1g.reshape(2, 128, C).transpose(1, 0, 2).reshape(128, 256).astype(bf)

    wbfb = np.zeros((128, WB_N), dtype=bf)
    wbfb[:, WB_W2:WB_W2 + 256] = \
        w2g.reshape(2, 128, C).transpose(1, 0, 2).reshape(128, 256).astype(bf)
    wbfb[:, WB_W24:WB_W24 + 128] = \
        w24.reshape(2, 128, O).transpose(1, 0, 2).reshape(128, 128).astype(bf)
    wbfb[:, WB_W4:WB_W4 + O] = w4f.astype(bf)

    wf32 = np.zeros((128, WF_N), dtype=np.float32)
    wf32[:, WF_W3:WF_W3 + O] = w3c
    wf32[:, WF_B1] = b1e
    wf32[:, WF_B2] = b2e
    wf32[0:O, WF_NB] = nbias

    ones = np.ones((4, ONES_N), dtype=bf)

    in_maps = []
    for k in range(NCORES):
        bi, q = k // (NCORES // B), k % (NCORES // B)
        xb = x[:, bi, :]                                    # (L, D)
        xtiles = xb.reshape(NT, 128, D)                     # (NT, 128, D)
        # rotate: slot s holds input tile (q+s)%NT; slot 0 = own l-block
        order = [(q + s) % NT for s in range(NT)]
        xall = np.ascontiguousarray(
            xtiles[order].transpose(1, 0, 2).astype(bf))    # (128, NT, D)
        in_maps.append({"xall": xall, "wbfa": wbfa, "wbfb": wbfb,
                        "wf32": wf32, "ones": ones})
    return in_maps


def kernel_run(inputs, trace=False):
    nc = _get_nc()
    in_maps = _make_in_maps(**inputs)
    res = run_bass_kernel_spmd(
        nc, in_maps, core_ids=list(range(NCORES)), trace=trace,
    )
    out = np.empty((B, L, L, O), dtype=np.float32)
    for k in range(NCORES):
        bi, q = k // (NCORES // B), k % (NCORES // B)
        blk = np.asarray(res.results[k]["out"]).astype(np.float32)
        tmp = blk.reshape(LBLK, O, NT, 128)      # (l, o, slot, j)
        rows = slice(q * LBLK, (q + 1) * LBLK)
        for s in range(NT):
            t = (q + s) % NT
            out[bi, rows, t * 128:(t + 1) * 128, :] = \
                tmp[:, :, s, :].transpose(0, 2, 1)
    return out, res


def kernel(**inputs) -> np.ndarray:
    out, _ = kernel_run(inputs, trace=False)
    return out


# revision 11
# speedup vs baseline: 1.0796x; 1.0796x over previous
"""Trainium2 Bass kernel for nn_ComparisonLayer (v4).

Computes, for x:(L,B,D) with L=512,B=2,D=256,C=128,O=64:
    xb  = layernorm(transpose(x,(1,0,2)))          # (B,L,D)
    a   = xb@w1+b1 ; b = xb@w2+b2                  # (B,L,C)
    out[b,l,m,o] = sum_c a[b,l,c]*b[b,m,c]*w3[c,o] + b3[o]
                   + (a@w4)[b,l,o] - (b@w4)[b,m,o] # (B,L,L,O)

Sharding: 8 cores, core k handles batch k//4 and l-block q=k%4.

Structure per core (all row terms folded into one K=2 rank matmul):
    out[l, o, m] = (aT * w3[:,o]).T @ bT           # main MM, K=128 N=512
                 + [ones; a4T[o]].T @ [negb4T[o]; ones]   # K=2 rank MM
Drains are pure PSUM->SBUF copies of 2-o (2-bank) packs, split ACT/DVE
5:3.  b4T comes directly from xn via host-folded w24 = w2g @ w4.  LN
stats use bn_stats/bn_aggr (one ACT table set, less ACT work).  The
ones planes for the rank matmuls and the warm-up operand are DMA'd
from DRAM.  One xall DMA (split DMAs block the in-order engine queues
on arrival sems).  Aggressive PE warm-up: the HAM clock gate needs
~3.4us of sustained matmul activity before the PE runs at 2.4 GHz.
Device output layout (l, (o, m_dev)) bf16; host un-rotates m (tiles
are rotated so slot 0 is the core's own l-block) and transposes to
(l, m, o) fp32.
"""

import numpy as np
import ml_dtypes

import concourse.bacc as bacc
import concourse.bass as bass
import concourse.mybir as mybir
import concourse.tile as tile
from concourse.bass_utils import run_bass_kernel_spmd

L, B, D, C, O = 512, 2, 256, 128, 64
NCORES = 8
LBLK = 128                   # l rows per core
NT = 4                       # row tiles of 128

F32 = mybir.dt.float32
BF16 = mybir.dt.bfloat16

# wbfa (bf16) columns: identity + w1 halves
WA_ID = 0            # [0:128)
WA_W1 = 128          # [128:384)  w1g halves (h p) c -> p (h c)
WA_N = 384
# wbfb (bf16) columns: w2 halves + w24 halves + w4
WB_W2 = 0            # [0:256)
WB_W24 = 256         # [256:384)  w24 halves (h p) o -> p (h o)
WB_W4 = 384          # [384:448)  w4 (C, O)
WB_N = 448
# wf32 columns
WF_W3 = 0            # [0:64)   w3 (C, O)
WF_B1 = 64           # b1e
WF_B2 = 65           # b2e
WF_NB = 66           # nbias = b3 - b2e@w4, partitions 0..63
WF_N = 67
# ones planes (DRAM): [4, 8192] rowRep rows 32r+1 + [4, 2048] rk rows 32r
ONES_N = 16 * L + 16 * 128   # 10240

# output blocks (o ranges per DMA)
BOUNDS = [0, 2, 4, 12, 20, 28, 36, 44, 52, 58, 62, 64]
OBLK_MAX = max(b - a for a, b in zip(BOUNDS, BOUNDS[1:]))


def _build():
    nc = bacc.Bacc("TRN2", target_bir_lowering=False, debug=False)

    xall_d = nc.dram_tensor("xall", (128, NT, D), BF16, kind="ExternalInput")
    wbfa_d = nc.dram_tensor("wbfa", (128, WA_N), BF16, kind="ExternalInput")
    wbfb_d = nc.dram_tensor("wbfb", (128, WB_N), BF16, kind="ExternalInput")
    wf32_d = nc.dram_tensor("wf32", (128, WF_N), F32, kind="ExternalInput")
    ones_d = nc.dram_tensor("ones", (4, ONES_N), BF16, kind="ExternalInput")
    out_d = nc.dram_tensor("out", (LBLK, O * L), BF16, kind="ExternalOutput")

    AX = mybir.AxisListType.X
    ALU = mybir.AluOpType
    ACT = mybir.ActivationFunctionType

    with tile.TileContext(nc) as tc:
        with (
            tc.tile_pool(name="const", bufs=1) as cp,
            tc.tile_pool(name="work", bufs=2) as wp,
            tc.tile_pool(name="aw", bufs=16) as awp,
            tc.tile_pool(name="ob", bufs=4) as obp,
            tc.tile_pool(name="ps", bufs=3, space="PSUM") as pm,
            tc.tile_pool(name="tp", bufs=1, space="PSUM") as tpm,
            tc.tile_pool(name="wf", bufs=1, space="PSUM") as wfm,
        ):
            # ---------- input DMAs (sync: x + ones planes; scalar: weights)
            xall = cp.tile([128, NT, D], BF16)
            wbfa = cp.tile([128, WA_N], BF16)
            wbfb = cp.tile([128, WB_N], BF16)
            wf32 = cp.tile([128, WF_N], F32)
            rowRep = cp.tile([128, 16 * L], BF16)
            rk = cp.tile([128, 16 * 128], BF16)
            nc.sync.dma_start(xall[:], xall_d[:])
            nc.sync.dma_start(wbfa[:], wbfa_d[:])
            nc.sync.dma_start(wbfb[:], wbfb_d[:])
            nc.sync.dma_start(wf32[:], wf32_d[:])
            nc.sync.dma_start(rowRep[1:128:32, :], ones_d[:, 0:16 * L])
            nc.sync.dma_start(rk[0:128:32, :], ones_d[:, 16 * L:ONES_N])

            id128 = wbfa[:, WA_ID:WA_ID + 128]
            w4s = wbfb[:, WB_W4:WB_W4 + O]

            onesb = cp.tile([128, 512], BF16)
            nc.vector.memset(onesb[:], 1.0)
            # dummy 1-col Sqrt pulls the ACT table load off the LN chain
            tblw = wp.tile([128, 1], F32, tag="tblw")
            nc.vector.memset(tblw[:], 1.0)
            nc.scalar.activation(tblw[:], tblw[:], ACT.Sqrt)

            # long-lived PSUM accumulators (allocate first: pin ring slots)
            bps_t = pm.tile([128, 2 * L], F32, tag="ps", name="bps")
            bps = bps_t[:, 0:L]
            b4ps_t = pm.tile([128, 2 * L], F32, tag="ps", name="b4ps")
            b4ps = b4ps_t[0:O, 0:L]

            # ---------- HAM warm-up ----------
            # The PE clock gate (HAM) needs ~3.4us of sustained matmul
            # activity to open to 2.4 GHz, and one mostly-idle window
            # re-throttles it; transposes do not count as activity.  Keep a
            # filler matmul stream through every PE idle window, via a
            # dedicated PSUM bank so fillers never contend with real work.
            wfill = wfm.tile([128, L], F32, tag="wf", name="wfill")

            def warm(n, nm):
                for wi in range(n):
                    nc.tensor.matmul(wfill[:], onesb[:, 0:128],
                                     onesb[:, 0:L], start=True, stop=True)

            warm(7, "wburst")

            # ---------- per-tile LN + transpose + bT/b4 matmuls ----------
            stat6 = wp.tile([128, NT, 6], F32, tag="stat6")
            mv = wp.tile([128, NT, 2], F32, tag="mv")
            rstd = wp.tile([128, NT], F32, tag="rstd")
            xn = cp.tile([128, NT, D], BF16)
            xnT = cp.tile([128, 2, NT, 128], BF16)
            aT_c = cp.tile([C, 128], BF16)
            a4bf = cp.tile([O, 128], BF16)
            bT_c = cp.tile([C, L], BF16)
            nb = cp.tile([O, L], BF16)

            aw_tiles = {}

            def build_aw4(k):  # o's [4k, 4k+4)
                # tile_wait_until paces the builds across the loop: the
                # scheduler otherwise hoists all of them into one long DVE
                # wall that starves the xnT-copy/negb4T/bT_c critical chain
                t = awp.tile([C, 4, 128], BF16, tag="aw", name=f"aw{k}")
                a_bc = aT_c[:].unsqueeze(1).broadcast_to((C, 4, 128))
                w_bc = wf32[:, WF_W3 + 4 * k:WF_W3 + 4 * k + 4] \
                    .unsqueeze(2).broadcast_to((C, 4, 128))
                with tc.tile_wait_until(0.012 + 0.0014 * k, enable=(k > 0)):
                    nc.vector.tensor_tensor(t[:], a_bc, w_bc, op=ALU.mult)
                aw_tiles[k] = t

            for s in range(NT):
                xs = xall[:, s, :]
                sc = slice(s, s + 1)
                nc.vector.bn_stats(stat6[:, s, :], xs)
                nc.vector.bn_aggr(mv[:, s, :], stat6[:, s, :])
                nc.scalar.activation(
                    rstd[:, sc], mv[:, s, 1:2], ACT.Sqrt)
                nc.vector.reciprocal(rstd[:, sc], rstd[:, sc])
                nc.vector.tensor_scalar(
                    xn[:, s, :], xs, mv[:, s, 0:1], rstd[:, sc],
                    op0=ALU.subtract, op1=ALU.mult)
                for h in range(2):
                    tp = tpm.tile([128, 512], BF16, tag="tp", name=f"tp{s}{h}")
                    nc.tensor.transpose(
                        tp[:, 0:128], xn[:, s, h * 128:(h + 1) * 128], id128)
                    if h == 0:
                        nc.scalar.copy(xnT[:, h, s, :], tp[:, 0:128])
                    else:
                        nc.vector.tensor_copy(xnT[:, h, s, :], tp[:, 0:128])
                # bT rows: accumulate w2g.T @ xnT into bps columns of tile s
                for h in range(2):
                    nc.tensor.matmul(
                        bps[:, s * 128:(s + 1) * 128],
                        wbfb[:, WB_W2 + h * 128:WB_W2 + (h + 1) * 128],
                        xnT[:, h, s, :],
                        start=(s == 0 and h == 0), stop=(s == 3 and h == 1),
                    )
                # b4T rows: w24.T @ xnT (direct, skips bT).  Per-tile
                # start/stop groups: start=True only clears has_written
                # bits, previously written column VALUES survive.
                for h in range(2):
                    nc.tensor.matmul(
                        b4ps[:, s * 128:(s + 1) * 128],
                        wbfb[:, WB_W24 + h * O:WB_W24 + (h + 1) * O],
                        xnT[:, h, s, :],
                        start=(h == 0), stop=(h == 1),
                    )
                # negb4T slice for this tile (gates rank matmuls)
                nc.scalar.activation(
                    nb[:, s * 128:(s + 1) * 128],
                    b4ps[:, s * 128:(s + 1) * 128], ACT.Identity,
                    bias=wf32[0:O, WF_NB:WF_NB + 1], scale=-1.0)
                if s == 0:
                    # aT / a4T chain (tile 0 == this core's own l-block)
                    aps_t = pm.tile([128, 2 * L], F32, tag="ps", name="aps")
                    for h in range(2):
                        nc.tensor.matmul(
                            aps_t[:, 0:128],
                            wbfa[:, WA_W1 + h * 128:WA_W1 + (h + 1) * 128],
                            xnT[:, h, 0, :], start=(h == 0), stop=(h == 1),
                        )
                    nc.vector.tensor_scalar_add(
                        aT_c[:], aps_t[:, 0:128], wf32[:, WF_B1:WF_B1 + 1])
                    a4ps_t = pm.tile([128, 2 * L], F32, tag="ps", name="a4ps")
                    nc.tensor.matmul(a4ps_t[0:O, 0:128], w4s, aT_c[:],
                                     start=True, stop=True)
                    nc.scalar.copy(a4bf[:], a4ps_t[0:O, 0:128])
                    # a4T strip replicas -> rk partitions 32r+1
                    for r in range(4):
                        eng = nc.sync if r % 2 == 0 else nc.scalar
                        eng.dma_start(
                            rk[32 * r + 1:32 * r + 2, :], a4bf[r:O:4, :])
                warm(2, f"wt{s}")

            # flatten negb4T strips into rowRep rows 32r; the g<4 columns
            # ship first (tiny transfers) so the first rank matmuls
            # unblock early, the rest follows under the running loop
            for r in range(4):
                eng = nc.sync if r % 2 == 0 else nc.scalar
                eng.dma_start(rowRep[32 * r:32 * r + 1, 0:4 * L],
                              nb[r:16:4, :])
            for r in range(4):
                eng = nc.sync if r % 2 == 0 else nc.scalar
                eng.dma_start(rowRep[32 * r:32 * r + 1, 4 * L:16 * L],
                              nb[16 + r:O:4, :])
            # bT_c (bias add) gates the main matmuls
            nc.vector.tensor_scalar_add(
                bT_c[:], bps[:], wf32[:, WF_B2:WF_B2 + 1])
            for k in range(3):
                build_aw4(k)
            # bridge the PE from the preamble into the main loop: the first
            # rank matmuls wait on the negb4T flatten chain
            warm(14, "wbridge")

            # ---------- main loop: 16 groups of 4 o's, 2-o PSUM packs ------
            blk_of = {}
            for bs, be in zip(BOUNDS, BOUNDS[1:]):
                for o in range(bs, be):
                    blk_of[o] = (bs, be)

            DRAIN_ACT = {0, 1, 3, 4, 6}   # p%8 -> ACT, else DVE (5:3)

            ob = None
            for g in range(O // 4):
                packA = pm.tile([128, 2 * L], F32, tag="ps", name=f"pkA{g}")
                packB = pm.tile([128, 2 * L], F32, tag="ps", name=f"pkB{g}")
                packs = (packA, packB)
                awb = aw_tiles[g]
                # main matmuls first (start=True)
                for j in range(4):
                    dst = packs[j // 2][:, (j % 2) * L:(j % 2 + 1) * L]
                    nc.tensor.matmul(dst, awb[:, j, :], bT_c[:],
                                     start=True, stop=False)
                # rank matmuls: [ones; a4T[o]] x [negb4T[o]; ones]
                for j in range(4):
                    dst = packs[j // 2][:, (j % 2) * L:(j % 2 + 1) * L]
                    nc.tensor.matmul(
                        dst,
                        rk[32 * j:32 * j + 2, g * 128:(g + 1) * 128],
                        rowRep[32 * j:32 * j + 2, g * L:(g + 1) * L],
                        start=False, stop=True, tile_position=(32 * j, 0),
                    )
                # drains: 2-o pure copies, ACT:DVE = 5:3
                for half in range(2):
                    p_idx = 2 * g + half
                    o0 = 2 * p_idx
                    bs, be = blk_of[o0]
                    if o0 == bs:
                        ob = obp.tile([128, OBLK_MAX * L], BF16, tag="ob",
                                      name=f"ob{p_idx}")
                    sl = ob[:, (o0 - bs) * L:(o0 - bs + 2) * L]
                    if p_idx % 8 in DRAIN_ACT:
                        nc.scalar.copy(sl, packs[half][:])
                    else:
                        nc.vector.tensor_copy(sl, packs[half][:])
                    if o0 + 2 == be:
                        nc.sync.dma_start(
                            out_d[:, bs * L:be * L], ob[:, 0:(be - bs) * L])
                # build one aw block ahead (after drains: drain priority)
                if g + 3 < O // 4:
                    build_aw4(g + 3)

    nc.compile()
    return nc


_CACHE = {}


def _get_nc():
    if "nc" not in _CACHE:
        _CACHE["nc"] = _build()
    return _CACHE["nc"]


def _make_in_maps(x, ln_gamma, ln_beta, w1, b1, w2, b2, w3, b3, w4):
    x = np.ascontiguousarray(x, dtype=np.float32)
    g = np.asarray(ln_gamma, np.float32)
    be = np.asarray(ln_beta, np.float32)
    w1 = np.asarray(w1, np.float32)
    w2 = np.asarray(w2, np.float32)
    # fold the LN affine into the first-layer weights:
    # (xn*g + be) @ w = xn @ (g[:,None]*w) + be @ w
    w1g = g[:, None] * w1
    w2g = g[:, None] * w2
    b1e = np.asarray(b1, np.float32) + be @ w1
    b2e = np.asarray(b2, np.float32) + be @ w2
    w3c = np.asarray(w3, np.float32)
    w4f = np.asarray(w4, np.float32)
    b3f = np.asarray(b3, np.float32)
    w24 = w2g @ w4f                      # (D, O)
    nbias = b3f - b2e @ w4f              # (O,)

    bf = ml_dtypes.bfloat16
    wbfa = np.zeros((128, WA_N), dtype=bf)
    wbfa[:, WA_ID:WA_ID + 128] = np.eye(128, dtype=np.float32).astype(bf)
    wbfa[:, WA_W1:WA_W1 + 256] = \
        w1g.reshape(2, 128, C).transpose(1, 0, 2).reshape(128, 256).astype(bf)

    wbfb = np.zeros((128, WB_N), dtype=bf)
    wbfb[:, WB_W2:WB_W2 + 256] = \
        w2g.reshape(2, 128, C).transpose(1, 0, 2).reshape(128, 256).astype(bf)
    wbfb[:, WB_W24:WB_W24 + 128] = \
        w24.reshape(2, 128, O).transpose(1, 0, 2).reshape(128, 128).astype(bf)
    wbfb[:, WB_W4:WB_W4 + O] = w4f.astype(bf)

    wf32 = np.zeros((128, WF_N), dtype=np.float32)
    wf32[:, WF_W3:WF_W3 + O] = w3c
    wf32[:, WF_B1] = b1e
    wf32[:, WF_B2] = b2e
    wf32[0:O, WF_NB] = nbias

    ones = np.ones((4, ONES_N), dtype=bf)

    in_maps = []
    for k in range(NCORES):
        bi, q = k // (NCORES // B), k % (NCORES // B)
        xb = x[:, bi, :]                                    # (L, D)
        xtiles = xb.reshape(NT, 128, D)                     # (NT, 128, D)
        # rotate: slot s holds input tile (q+s)%NT; slot 0 = own l-block
        order = [(q + s) % NT for s in range(NT)]
        xall = np.ascontiguousarray(
            xtiles[order].transpose(1, 0, 2).astype(bf))    # (128, NT, D)
        in_maps.append({"xall": xall, "wbfa": wbfa, "wbfb": wbfb,
                        "wf32": wf32, "ones": ones})
    return in_maps


def kernel_run(inputs, trace=False):
    nc = _get_nc()
    in_maps = _make_in_maps(**inputs)
    res = run_bass_kernel_spmd(
        nc, in_maps, core_ids=list(range(NCORES)), trace=trace,
    )
    out = np.empty((B, L, L, O), dtype=np.float32)
    for k in range(NCORES):
        bi, q = k // (NCORES // B), k % (NCORES // B)
        blk = np.asarray(res.results[k]["out"]).astype(np.float32)
        tmp = blk.reshape(LBLK, O, NT, 128)      # (l, o, slot, j)
        rows = slice(q * LBLK, (q + 1) * LBLK)
        for s in range(NT):
            t = (q + s) % NT
            out[bi, rows, t * 128:(t + 1) * 128, :] = \
                tmp[:, :, s, :].transpose(0, 2, 1)
    return out, res


def kernel(**inputs) -> np.ndarray:
    out, _ = kernel_run(inputs, trace=False)
    return out


# revision 12
# speedup vs baseline: 1.1114x; 1.0295x over previous
"""Trainium2 Bass kernel for nn_ComparisonLayer (v4).

Computes, for x:(L,B,D) with L=512,B=2,D=256,C=128,O=64:
    xb  = layernorm(transpose(x,(1,0,2)))          # (B,L,D)
    a   = xb@w1+b1 ; b = xb@w2+b2                  # (B,L,C)
    out[b,l,m,o] = sum_c a[b,l,c]*b[b,m,c]*w3[c,o] + b3[o]
                   + (a@w4)[b,l,o] - (b@w4)[b,m,o] # (B,L,L,O)

Sharding: 8 cores, core k handles batch k//4 and l-block q=k%4.

Structure per core (all row terms folded into one K=2 rank matmul):
    out[l, o, m] = (aT * w3[:,o]).T @ bT           # main MM, K=128 N=512
                 + [ones; a4T[o]].T @ [negb4T[o]; ones]   # K=2 rank MM
Drains are pure PSUM->SBUF copies of 2-o (2-bank) packs, split ACT/DVE
5:3.  b4T comes directly from xn via host-folded w24 = w2g @ w4.  LN
stats use bn_stats/bn_aggr (one ACT table set, less ACT work).  The
ones planes for the rank matmuls and the warm-up operand are DMA'd
from DRAM.  One xall DMA (split DMAs block the in-order engine queues
on arrival sems).  Aggressive PE warm-up: the HAM clock gate needs
~3.4us of sustained matmul activity before the PE runs at 2.4 GHz.
Device output layout (l, (o, m_dev)) bf16; host un-rotates m (tiles
are rotated so slot 0 is the core's own l-block) and transposes to
(l, m, o) fp32.
"""

import numpy as np
import ml_dtypes

import concourse.bacc as bacc
import concourse.bass as bass
import concourse.mybir as mybir
import concourse.tile as tile
from concourse.bass_utils import run_bass_kernel_spmd

L, B, D, C, O = 512, 2, 256, 128, 64
NCORES = 8
LBLK = 128                   # l rows per core
NT = 4                       # row tiles of 128

F32 = mybir.dt.float32
BF16 = mybir.dt.bfloat16

# wbfa (bf16) columns: identity + w1 halves
WA_ID = 0            # [0:128)
WA_W1 = 128          # [128:384)  w1g halves (h p) c -> p (h c)
WA_N = 384
# wbfb (bf16) columns: w2 halves + w24 halves + w4
WB_W2 = 0            # [0:256)
WB_W24 = 256         # [256:384)  w24 halves (h p) o -> p (h o)
WB_W4 = 384          # [384:448)  w4 (C, O)
WB_N = 448
# wf32 columns
WF_W3 = 0            # [0:64)   w3 (C, O)
WF_B1 = 64           # b1e
WF_B2 = 65           # b2e
WF_NB = 66           # nbias = b3 - b2e@w4, partitions 0..63
WF_N = 67
# ones planes (DRAM): [4, 8192] rowRep rows 32r+1 + [4, 2048] rk rows 32r
ONES_N = 16 * L + 16 * 128   # 10240

# output blocks (o ranges per DMA)
BOUNDS = [0, 2, 4, 12, 20, 28, 36, 44, 52, 58, 62, 64]
OBLK_MAX = max(b - a for a, b in zip(BOUNDS, BOUNDS[1:]))


def _build():
    nc = bacc.Bacc("TRN2", target_bir_lowering=False, debug=False)

    xall_d = nc.dram_tensor("xall", (128, NT, D), BF16, kind="ExternalInput")
    wbfa_d = nc.dram_tensor("wbfa", (128, WA_N), BF16, kind="ExternalInput")
    wbfb_d = nc.dram_tensor("wbfb", (128, WB_N), BF16, kind="ExternalInput")
    wf32_d = nc.dram_tensor("wf32", (128, WF_N), F32, kind="ExternalInput")
    ones_d = nc.dram_tensor("ones", (4, ONES_N), BF16, kind="ExternalInput")
    out_d = nc.dram_tensor("out", (LBLK, O * L), BF16, kind="ExternalOutput")

    AX = mybir.AxisListType.X
    ALU = mybir.AluOpType
    ACT = mybir.ActivationFunctionType

    with tile.TileContext(nc) as tc:
        with (
            tc.tile_pool(name="const", bufs=1) as cp,
            tc.tile_pool(name="work", bufs=2) as wp,
            tc.tile_pool(name="aw", bufs=16) as awp,
            tc.tile_pool(name="ob", bufs=4) as obp,
            tc.tile_pool(name="ps", bufs=3, space="PSUM") as pm,
            tc.tile_pool(name="tp", bufs=1, space="PSUM") as tpm,
            tc.tile_pool(name="wf", bufs=1, space="PSUM") as wfm,
        ):
            # ---------- input DMAs (sync: x + ones planes; scalar: weights)
            xall = cp.tile([128, NT, D], BF16)
            wbfa = cp.tile([128, WA_N], BF16)
            wbfb = cp.tile([128, WB_N], BF16)
            wf32 = cp.tile([128, WF_N], F32)
            rowRep = cp.tile([128, 16 * L], BF16)
            rk = cp.tile([128, 16 * 128], BF16)
            nc.sync.dma_start(xall[:], xall_d[:])
            nc.sync.dma_start(wbfa[:], wbfa_d[:])
            nc.sync.dma_start(wbfb[:], wbfb_d[:])
            nc.sync.dma_start(wf32[:], wf32_d[:])
            nc.sync.dma_start(rowRep[1:128:32, :], ones_d[:, 0:16 * L])
            nc.sync.dma_start(rk[0:128:32, :], ones_d[:, 16 * L:ONES_N])

            id128 = wbfa[:, WA_ID:WA_ID + 128]
            w4s = wbfb[:, WB_W4:WB_W4 + O]

            onesb = cp.tile([128, 512], BF16)
            nc.vector.memset(onesb[:], 1.0)
            # dummy 1-col Sqrt pulls the ACT table load off the LN chain
            tblw = wp.tile([128, 1], F32, tag="tblw")
            nc.vector.memset(tblw[:], 1.0)
            nc.scalar.activation(tblw[:], tblw[:], ACT.Sqrt)

            # long-lived PSUM accumulators (allocate first: pin ring slots)
            bps_t = pm.tile([128, 2 * L], F32, tag="ps", name="bps")
            bps = bps_t[:, 0:L]
            b4ps_t = pm.tile([128, 2 * L], F32, tag="ps", name="b4ps")
            b4ps = b4ps_t[0:O, 0:L]

            # ---------- HAM warm-up ----------
            # The PE clock gate (HAM) needs ~3.4us of sustained matmul
            # activity to open to 2.4 GHz, and one mostly-idle window
            # re-throttles it; transposes do not count as activity.  Keep a
            # filler matmul stream through every PE idle window, via a
            # dedicated PSUM bank so fillers never contend with real work.
            wfill = wfm.tile([128, L], F32, tag="wf", name="wfill")

            def warm(n, nm):
                for wi in range(n):
                    nc.tensor.matmul(wfill[:], onesb[:, 0:128],
                                     onesb[:, 0:L], start=True, stop=True)

            warm(7, "wburst")

            # ---------- per-tile LN + transpose + bT/b4 matmuls ----------
            stat6 = wp.tile([128, NT, 6], F32, tag="stat6")
            mv = wp.tile([128, NT, 2], F32, tag="mv")
            rstd = wp.tile([128, NT], F32, tag="rstd")
            xn = cp.tile([128, NT, D], BF16)
            xnT = cp.tile([128, 2, NT, 128], BF16)
            aT_c = cp.tile([C, 128], BF16)
            a4bf = cp.tile([O, 128], BF16)
            bT_c = cp.tile([C, L], BF16)
            nb = cp.tile([O, L], BF16)

            aw_tiles = {}

            def build_aw4(k):  # o's [4k, 4k+4)
                # tile_wait_until paces the builds across the loop: the
                # scheduler otherwise hoists all of them into one long DVE
                # wall that starves the xnT-copy/negb4T/bT_c critical chain
                t = awp.tile([C, 4, 128], BF16, tag="aw", name=f"aw{k}")
                a_bc = aT_c[:].unsqueeze(1).broadcast_to((C, 4, 128))
                w_bc = wf32[:, WF_W3 + 4 * k:WF_W3 + 4 * k + 4] \
                    .unsqueeze(2).broadcast_to((C, 4, 128))
                with tc.tile_wait_until(0.012 + 0.0014 * k, enable=(k > 0)):
                    nc.vector.tensor_tensor(t[:], a_bc, w_bc, op=ALU.mult)
                aw_tiles[k] = t

            for s in range(NT):
                xs = xall[:, s, :]
                sc = slice(s, s + 1)
                nc.vector.bn_stats(stat6[:, s, :], xs)
                nc.vector.bn_aggr(mv[:, s, :], stat6[:, s, :])
                nc.scalar.activation(
                    rstd[:, sc], mv[:, s, 1:2], ACT.Sqrt)
                nc.vector.reciprocal(rstd[:, sc], rstd[:, sc])
                nc.vector.tensor_scalar(
                    xn[:, s, :], xs, mv[:, s, 0:1], rstd[:, sc],
                    op0=ALU.subtract, op1=ALU.mult)
                for h in range(2):
                    tp = tpm.tile([128, 512], BF16, tag="tp", name=f"tp{s}{h}")
                    nc.tensor.transpose(
                        tp[:, 0:128], xn[:, s, h * 128:(h + 1) * 128], id128)
                    if h == 0:
                        nc.scalar.copy(xnT[:, h, s, :], tp[:, 0:128])
                    else:
                        nc.vector.tensor_copy(xnT[:, h, s, :], tp[:, 0:128])
                # bT rows: accumulate w2g.T @ xnT into bps columns of tile s
                for h in range(2):
                    nc.tensor.matmul(
                        bps[:, s * 128:(s + 1) * 128],
                        wbfb[:, WB_W2 + h * 128:WB_W2 + (h + 1) * 128],
                        xnT[:, h, s, :],
                        start=(s == 0 and h == 0), stop=(s == 3 and h == 1),
                    )
                # b4T rows: w24.T @ xnT (direct, skips bT).  Per-tile
                # start/stop groups: start=True only clears has_written
                # bits, previously written column VALUES survive.
                for h in range(2):
                    nc.tensor.matmul(
                        b4ps[:, s * 128:(s + 1) * 128],
                        wbfb[:, WB_W24 + h * O:WB_W24 + (h + 1) * O],
                        xnT[:, h, s, :],
                        start=(h == 0), stop=(h == 1),
                    )
                # negb4T slice for this tile (gates rank matmuls)
                nc.scalar.activation(
                    nb[:, s * 128:(s + 1) * 128],
                    b4ps[:, s * 128:(s + 1) * 128], ACT.Identity,
                    bias=wf32[0:O, WF_NB:WF_NB + 1], scale=-1.0)
                if s == 0:
                    # aT / a4T chain (tile 0 == this core's own l-block)
                    aps_t = pm.tile([128, 2 * L], F32, tag="ps", name="aps")
                    for h in range(2):
                        nc.tensor.matmul(
                            aps_t[:, 0:128],
                            wbfa[:, WA_W1 + h * 128:WA_W1 + (h + 1) * 128],
                            xnT[:, h, 0, :], start=(h == 0), stop=(h == 1),
                        )
                    nc.vector.tensor_scalar_add(
                        aT_c[:], aps_t[:, 0:128], wf32[:, WF_B1:WF_B1 + 1])
                    a4ps_t = pm.tile([128, 2 * L], F32, tag="ps", name="a4ps")
                    nc.tensor.matmul(a4ps_t[0:O, 0:128], w4s, aT_c[:],
                                     start=True, stop=True)
                    nc.scalar.copy(a4bf[:], a4ps_t[0:O, 0:128])
                    # a4T strip replicas -> rk partitions 32r+1
                    for r in range(4):
                        nc.sync.dma_start(
                            rk[32 * r + 1:32 * r + 2, :], a4bf[r:O:4, :])
                warm(1, f"wt{s}")

            # flatten negb4T strips into rowRep rows 32r; the g<4 columns
            # ship first (tiny transfers) so the first rank matmuls
            # unblock early, the rest follows under the running loop
            for r in range(4):
                nc.sync.dma_start(rowRep[32 * r:32 * r + 1, 0:4 * L],
                                  nb[r:16:4, :])
            for r in range(4):
                nc.sync.dma_start(rowRep[32 * r:32 * r + 1, 4 * L:16 * L],
                                  nb[16 + r:O:4, :])
            # bT_c (bias add) gates the main matmuls
            nc.vector.tensor_scalar_add(
                bT_c[:], bps[:], wf32[:, WF_B2:WF_B2 + 1])
            for k in range(3):
                build_aw4(k)
            # short bridge; the big filler run sits between group 0's main
            # and rank matmuls (the ranks wait on the negb4T flatten chain,
            # and a cold loop at ~80% PE-busy never re-warms the HAM)
            warm(6, "wbridge")

            # ---------- main loop: 16 groups of 4 o's, 2-o PSUM packs ------
            blk_of = {}
            for bs, be in zip(BOUNDS, BOUNDS[1:]):
                for o in range(bs, be):
                    blk_of[o] = (bs, be)

            DRAIN_ACT = {0, 1, 3, 4, 6}   # p%8 -> ACT, else DVE (5:3)

            ob = None
            for g in range(O // 4):
                packA = pm.tile([128, 2 * L], F32, tag="ps", name=f"pkA{g}")
                packB = pm.tile([128, 2 * L], F32, tag="ps", name=f"pkB{g}")
                packs = (packA, packB)
                awb = aw_tiles[g]
                # main matmuls first (start=True)
                for j in range(4):
                    dst = packs[j // 2][:, (j % 2) * L:(j % 2 + 1) * L]
                    nc.tensor.matmul(dst, awb[:, j, :], bT_c[:],
                                     start=True, stop=False)
                if g == 0:
                    warm(34, "wgap0")
                elif g == 1:
                    warm(4, "wgap1")
                elif g == 2:
                    warm(2, "wgap2")
                # rank matmuls: [ones; a4T[o]] x [negb4T[o]; ones]
                for j in range(4):
                    dst = packs[j // 2][:, (j % 2) * L:(j % 2 + 1) * L]
                    nc.tensor.matmul(
                        dst,
                        rk[32 * j:32 * j + 2, g * 128:(g + 1) * 128],
                        rowRep[32 * j:32 * j + 2, g * L:(g + 1) * L],
                        start=False, stop=True, tile_position=(32 * j, 0),
                    )
                # drains: 2-o pure copies, ACT:DVE = 5:3
                for half in range(2):
                    p_idx = 2 * g + half
                    o0 = 2 * p_idx
                    bs, be = blk_of[o0]
                    if o0 == bs:
                        ob = obp.tile([128, OBLK_MAX * L], BF16, tag="ob",
                                      name=f"ob{p_idx}")
                    sl = ob[:, (o0 - bs) * L:(o0 - bs + 2) * L]
                    if p_idx % 8 in DRAIN_ACT:
                        nc.scalar.copy(sl, packs[half][:])
                    else:
                        nc.vector.tensor_copy(sl, packs[half][:])
                    if o0 + 2 == be:
                        nc.sync.dma_start(
                            out_d[:, bs * L:be * L], ob[:, 0:(be - bs) * L])
                # build one aw block ahead (after drains: drain priority)
                if g + 3 < O // 4:
                    build_aw4(g + 3)

    nc.compile()
    return nc


_CACHE = {}


def _get_nc():
    if "nc" not in _CACHE:
        _CACHE["nc"] = _build()
    return _CACHE["nc"]


def _make_in_maps(x, ln_gamma, ln_beta, w1, b1, w2, b2, w3, b3, w4):
    x = np.ascontiguousarray(x, dtype=np.float32)
    g = np.asarray(ln_gamma, np.float32)
    be = np.asarray(ln_beta, np.float32)
    w1 = np.asarray(w1, np.float32)
    w2 = np.asarray(w2, np.float32)
    # fold the LN affine into the first-layer weights:
    # (xn*g + be) @ w = xn @ (g[:,None]*w) + be @ w
    w1g = g[:, None] * w1
    w2g = g[:, None] * w2
    b1e = np.asarray(b1, np.float32) + be @ w1
    b2e = np.asarray(b2, np.float32) + be @ w2
    w3c = np.asarray(w3, np.float32)
    w4f = np.asarray(w4, np.float32)
    b3f = np.asarray(b3, np.float32)
    w24 = w2g @ w4f                      # (D, O)
    nbias = b3f - b2e @ w4f              # (O,)

    bf = ml_dtypes.bfloat16
    wbfa = np.zeros((128, WA_N), dtype=bf)
    wbfa[:, WA_ID:WA_ID + 128] = np.eye(128, dtype=np.float32).astype(bf)
    wbfa[:, WA_W1:WA_W1 + 256] = \
        w1g.reshape(2, 128, C).transpose(1, 0, 2).reshape(128, 256).astype(bf)

    wbfb = np.zeros((128, WB_N), dtype=bf)
    wbfb[:, WB_W2:WB_W2 + 256] = \
        w2g.reshape(2, 128, C).transpose(1, 0, 2).reshape(128, 256).astype(bf)
    wbfb[:, WB_W24:WB_W24 + 128] = \
        w24.reshape(2, 128, O).transpose(1, 0, 2).reshape(128, 128).astype(bf)
    wbfb[:, WB_W4:WB_W4 + O] = w4f.astype(bf)

    wf32 = np.zeros((128, WF_N), dtype=np.float32)
    wf32[:, WF_W3:WF_W3 + O] = w3c
    wf32[:, WF_B1] = b1e
    wf32[:, WF_B2] = b2e
    wf32[0:O, WF_NB] = nbias

    ones = np.ones((4, ONES_N), dtype=bf)

    in_maps = []
    for k in range(NCORES):
        bi, q = k // (NCORES // B), k % (NCORES // B)
        xb = x[:, bi, :]                                    # (L, D)
        xtiles = xb.reshape(NT, 128, D)                     # (NT, 128, D)
        # rotate: slot s holds input tile (q+s)%NT; slot 0 = own l-block
        order = [(q + s) % NT for s in range(NT)]
        xall = np.ascontiguousarray(
            xtiles[order].transpose(1, 0, 2).astype(bf))    # (128, NT, D)
        in_maps.append({"xall": xall, "wbfa": wbfa, "wbfb": wbfb,
                        "wf32": wf32, "ones": ones})
    return in_maps


def kernel_run(inputs, trace=False):
    nc = _get_nc()
    in_maps = _make_in_maps(**inputs)
    res = run_bass_kernel_spmd(
        nc, in_maps, core_ids=list(range(NCORES)), trace=trace,
    )
    out = np.empty((B, L, L, O), dtype=np.float32)
    for k in range(NCORES):
        bi, q = k // (NCORES // B), k % (NCORES // B)
        blk = np.asarray(res.results[k]["out"]).astype(np.float32)
        tmp = blk.reshape(LBLK, O, NT, 128)      # (l, o, slot, j)
        rows = slice(q * LBLK, (q + 1) * LBLK)
        for s in range(NT):
            t = (q + s) % NT
            out[bi, rows, t * 128:(t + 1) * 128, :] = \
                tmp[:, :, s, :].transpose(0, 2, 1)
    return out, res


def kernel(**inputs) -> np.ndarray:
    out, _ = kernel_run(inputs, trace=False)
    return out


# revision 13
# speedup vs baseline: 1.2057x; 1.0848x over previous
"""Trainium2 Bass kernel for nn_ComparisonLayer (v4).

Computes, for x:(L,B,D) with L=512,B=2,D=256,C=128,O=64:
    xb  = layernorm(transpose(x,(1,0,2)))          # (B,L,D)
    a   = xb@w1+b1 ; b = xb@w2+b2                  # (B,L,C)
    out[b,l,m,o] = sum_c a[b,l,c]*b[b,m,c]*w3[c,o] + b3[o]
                   + (a@w4)[b,l,o] - (b@w4)[b,m,o] # (B,L,L,O)

Sharding: 8 cores, core k handles batch k//4 and l-block q=k%4.

Structure per core (all row terms folded into one K=2 rank matmul):
    out[l, o, m] = (aT * w3[:,o]).T @ bT           # main MM, K=128 N=512
                 + [ones; a4T[o]].T @ [negb4T[o]; ones]   # K=2 rank MM
Drains are pure PSUM->SBUF copies of 2-o (2-bank) packs, split ACT/DVE
5:3.  b4T comes directly from xn via host-folded w24 = w2g @ w4.  LN
stats use bn_stats/bn_aggr (one ACT table set, less ACT work).  The
ones planes for the rank matmuls and the warm-up operand are DMA'd
from DRAM.  One xall DMA (split DMAs block the in-order engine queues
on arrival sems).  Aggressive PE warm-up: the HAM clock gate needs
~3.4us of sustained matmul activity before the PE runs at 2.4 GHz.
Device output layout (l, (o, m_dev)) bf16; host un-rotates m (tiles
are rotated so slot 0 is the core's own l-block) and transposes to
(l, m, o) fp32.
"""

import numpy as np
import ml_dtypes

import concourse.bacc as bacc
import concourse.bass as bass
import concourse.mybir as mybir
import concourse.tile as tile
from concourse.bass_utils import run_bass_kernel_spmd

L, B, D, C, O = 512, 2, 256, 128, 64
NCORES = 8
LBLK = 128                   # l rows per core
NT = 4                       # row tiles of 128

F32 = mybir.dt.float32
BF16 = mybir.dt.bfloat16

# wbfa (bf16) columns: identity + w1 halves
WA_ID = 0            # [0:128)
WA_W1 = 128          # [128:384)  w1g halves (h p) c -> p (h c)
WA_N = 384
# wbfb (bf16) columns: w2 halves + w24 halves + w4
WB_W2 = 0            # [0:256)
WB_W24 = 256         # [256:384)  w24 halves (h p) o -> p (h o)
WB_W4 = 384          # [384:448)  w4 (C, O)
WB_N = 448
# wf32 columns
WF_W3 = 0            # [0:64)   w3 (C, O)
WF_B1 = 64           # b1e
WF_B2 = 65           # b2e
WF_NB = 66           # nbias = b3 - b2e@w4, partitions 0..63
WF_N = 67
# ones planes (DRAM): [4, 8192] rowRep rows 32r+1 + [4, 2048] rk rows 32r
ONES_N = 16 * L + 16 * 128   # 10240

# output blocks (o ranges per DMA)
BOUNDS = [0, 2, 4, 12, 20, 28, 36, 44, 52, 58, 62, 64]
OBLK_MAX = max(b - a for a, b in zip(BOUNDS, BOUNDS[1:]))


def _build():
    nc = bacc.Bacc("TRN2", target_bir_lowering=False, debug=False)

    xall_d = nc.dram_tensor("xall", (128, NT, D), BF16, kind="ExternalInput")
    wbfa_d = nc.dram_tensor("wbfa", (128, WA_N), BF16, kind="ExternalInput")
    wbfb_d = nc.dram_tensor("wbfb", (128, WB_N), BF16, kind="ExternalInput")
    wf32_d = nc.dram_tensor("wf32", (128, WF_N), F32, kind="ExternalInput")
    ones_d = nc.dram_tensor("ones", (4, ONES_N), BF16, kind="ExternalInput")
    out_d = nc.dram_tensor("out", (LBLK, O * L), BF16, kind="ExternalOutput")

    AX = mybir.AxisListType.X
    ALU = mybir.AluOpType
    ACT = mybir.ActivationFunctionType

    with tile.TileContext(nc) as tc:
        with (
            tc.tile_pool(name="const", bufs=1) as cp,
            tc.tile_pool(name="work", bufs=2) as wp,
            tc.tile_pool(name="aw", bufs=16) as awp,
            tc.tile_pool(name="ob", bufs=4) as obp,
            tc.tile_pool(name="ps", bufs=3, space="PSUM") as pm,
            tc.tile_pool(name="tp", bufs=1, space="PSUM") as tpm,
            tc.tile_pool(name="wf", bufs=1, space="PSUM") as wfm,
        ):
            # ---------- input DMAs (sync: x + ones planes; scalar: weights)
            xall = cp.tile([128, NT, D], BF16)
            wbfa = cp.tile([128, WA_N], BF16)
            wbfb = cp.tile([128, WB_N], BF16)
            wf32 = cp.tile([128, WF_N], F32)
            rowRep = cp.tile([128, 16 * L], BF16)
            rk = cp.tile([128, 16 * 128], BF16)
            nc.sync.dma_start(xall[:], xall_d[:])
            nc.sync.dma_start(wbfa[:], wbfa_d[:])
            nc.sync.dma_start(wbfb[:], wbfb_d[:])
            nc.sync.dma_start(wf32[:], wf32_d[:])
            nc.sync.dma_start(rowRep[1:128:32, :], ones_d[:, 0:16 * L])
            nc.sync.dma_start(rk[0:128:32, :], ones_d[:, 16 * L:ONES_N])

            id128 = wbfa[:, WA_ID:WA_ID + 128]
            w4s = wbfb[:, WB_W4:WB_W4 + O]

            onesb = cp.tile([128, 512], BF16)
            nc.vector.memset(onesb[:], 1.0)
            # dummy 1-col Sqrt pulls the ACT table load off the LN chain
            tblw = wp.tile([128, 1], F32, tag="tblw")
            nc.vector.memset(tblw[:], 1.0)
            nc.scalar.activation(tblw[:], tblw[:], ACT.Sqrt)

            # long-lived PSUM accumulators (allocate first: pin ring slots)
            bps_t = pm.tile([128, 2 * L], F32, tag="ps", name="bps")
            bps = bps_t[:, 0:L]
            b4ps_t = pm.tile([128, 2 * L], F32, tag="ps", name="b4ps")
            b4ps = b4ps_t[0:O, 0:L]

            # ---------- HAM warm-up ----------
            # The PE clock gate (HAM) needs ~3.4us of sustained matmul
            # activity to open to 2.4 GHz, and one mostly-idle window
            # re-throttles it; transposes do not count as activity.  Keep a
            # filler matmul stream through every PE idle window, via a
            # dedicated PSUM bank so fillers never contend with real work.
            wfill = wfm.tile([128, L], F32, tag="wf", name="wfill")

            def warm(n, nm):
                for wi in range(n):
                    nc.tensor.matmul(wfill[:], onesb[:, 0:128],
                                     onesb[:, 0:L], start=True, stop=True)

            warm(7, "wburst")

            # ---------- per-tile LN + transpose + bT/b4 matmuls ----------
            stat6 = wp.tile([128, NT, 6], F32, tag="stat6")
            mv = wp.tile([128, NT, 2], F32, tag="mv")
            rstd = wp.tile([128, NT], F32, tag="rstd")
            xn = cp.tile([128, NT, D], BF16)
            xnT = cp.tile([128, 2, NT, 128], BF16)
            aT_c = cp.tile([C, 128], BF16)
            a4bf = cp.tile([O, 128], BF16)
            bT_c = cp.tile([C, L], BF16)
            nb = cp.tile([O, L], BF16)

            aw_tiles = {}

            def build_aw4(k):  # o's [4k, 4k+4)
                t = awp.tile([C, 4, 128], BF16, tag="aw", name=f"aw{k}")
                a_bc = aT_c[:].unsqueeze(1).broadcast_to((C, 4, 128))
                w_bc = wf32[:, WF_W3 + 4 * k:WF_W3 + 4 * k + 4] \
                    .unsqueeze(2).broadcast_to((C, 4, 128))
                nc.vector.tensor_tensor(t[:], a_bc, w_bc, op=ALU.mult)
                aw_tiles[k] = t

            for s in range(NT):
                xs = xall[:, s, :]
                sc = slice(s, s + 1)
                nc.vector.bn_stats(stat6[:, s, :], xs)
                nc.vector.bn_aggr(mv[:, s, :], stat6[:, s, :])
                nc.scalar.activation(
                    rstd[:, sc], mv[:, s, 1:2], ACT.Sqrt)
                nc.vector.reciprocal(rstd[:, sc], rstd[:, sc])
                nc.vector.tensor_scalar(
                    xn[:, s, :], xs, mv[:, s, 0:1], rstd[:, sc],
                    op0=ALU.subtract, op1=ALU.mult)
                for h in range(2):
                    tp = tpm.tile([128, 512], BF16, tag="tp", name=f"tp{s}{h}")
                    nc.tensor.transpose(
                        tp[:, 0:128], xn[:, s, h * 128:(h + 1) * 128], id128)
                    if h == 0:
                        nc.scalar.copy(xnT[:, h, s, :], tp[:, 0:128])
                    else:
                        nc.vector.tensor_copy(xnT[:, h, s, :], tp[:, 0:128])
                # bT rows: accumulate w2g.T @ xnT into bps columns of tile s
                for h in range(2):
                    nc.tensor.matmul(
                        bps[:, s * 128:(s + 1) * 128],
                        wbfb[:, WB_W2 + h * 128:WB_W2 + (h + 1) * 128],
                        xnT[:, h, s, :],
                        start=(s == 0 and h == 0), stop=(s == 3 and h == 1),
                    )
                # b4T rows: w24.T @ xnT (direct, skips bT).  Per-tile
                # start/stop groups: start=True only clears has_written
                # bits, previously written column VALUES survive.
                for h in range(2):
                    nc.tensor.matmul(
                        b4ps[:, s * 128:(s + 1) * 128],
                        wbfb[:, WB_W24 + h * O:WB_W24 + (h + 1) * O],
                        xnT[:, h, s, :],
                        start=(h == 0), stop=(h == 1),
                    )
                # negb4T slice for this tile (gates rank matmuls)
                nc.scalar.activation(
                    nb[:, s * 128:(s + 1) * 128],
                    b4ps[:, s * 128:(s + 1) * 128], ACT.Identity,
                    bias=wf32[0:O, WF_NB:WF_NB + 1], scale=-1.0)
                if s == 0:
                    # aT / a4T chain (tile 0 == this core's own l-block)
                    aps_t = pm.tile([128, 2 * L], F32, tag="ps", name="aps")
                    for h in range(2):
                        nc.tensor.matmul(
                            aps_t[:, 0:128],
                            wbfa[:, WA_W1 + h * 128:WA_W1 + (h + 1) * 128],
                            xnT[:, h, 0, :], start=(h == 0), stop=(h == 1),
                        )
                    nc.vector.tensor_scalar_add(
                        aT_c[:], aps_t[:, 0:128], wf32[:, WF_B1:WF_B1 + 1])
                    a4ps_t = pm.tile([128, 2 * L], F32, tag="ps", name="a4ps")
                    nc.tensor.matmul(a4ps_t[0:O, 0:128], w4s, aT_c[:],
                                     start=True, stop=True)
                    nc.scalar.copy(a4bf[:], a4ps_t[0:O, 0:128])
                    # a4T strip replicas -> rk partitions 32r+1
                    for r in range(4):
                        nc.sync.dma_start(
                            rk[32 * r + 1:32 * r + 2, :], a4bf[r:O:4, :])
                warm(2, f"wt{s}")

            # flatten negb4T strips into rowRep rows 32r; the g<4 columns
            # ship first (tiny transfers) so the first rank matmuls
            # unblock early, the rest follows under the running loop
            for r in range(4):
                nc.sync.dma_start(rowRep[32 * r:32 * r + 1, 0:4 * L],
                                  nb[r:16:4, :])
            for r in range(4):
                nc.sync.dma_start(rowRep[32 * r:32 * r + 1, 4 * L:16 * L],
                                  nb[16 + r:O:4, :])
            # bT_c (bias add) gates the main matmuls
            nc.vector.tensor_scalar_add(
                bT_c[:], bps[:], wf32[:, WF_B2:WF_B2 + 1])
            for k in range(3):
                build_aw4(k)
            # bridge the PE from the preamble into the main loop: the first
            # rank matmuls wait on the negb4T flatten chain
            warm(14, "wbridge")

            # ---------- main loop: 16 groups of 4 o's, 2-o PSUM packs ------
            blk_of = {}
            for bs, be in zip(BOUNDS, BOUNDS[1:]):
                for o in range(bs, be):
                    blk_of[o] = (bs, be)

            DRAIN_ACT = {0, 1, 3, 4, 6}   # p%8 -> ACT, else DVE (5:3)

            ob = None
            for g in range(O // 4):
                packA = pm.tile([128, 2 * L], F32, tag="ps", name=f"pkA{g}")
                packB = pm.tile([128, 2 * L], F32, tag="ps", name=f"pkB{g}")
                packs = (packA, packB)
                awb = aw_tiles[g]
                # main matmuls first (start=True)
                for j in range(4):
                    dst = packs[j // 2][:, (j % 2) * L:(j % 2 + 1) * L]
                    nc.tensor.matmul(dst, awb[:, j, :], bT_c[:],
                                     start=True, stop=False)
                # rank matmuls: [ones; a4T[o]] x [negb4T[o]; ones]
                for j in range(4):
                    dst = packs[j // 2][:, (j % 2) * L:(j % 2 + 1) * L]
                    nc.tensor.matmul(
                        dst,
                        rk[32 * j:32 * j + 2, g * 128:(g + 1) * 128],
                        rowRep[32 * j:32 * j + 2, g * L:(g + 1) * L],
                        start=False, stop=True, tile_position=(32 * j, 0),
                    )
                # drains: 2-o pure copies, ACT:DVE = 5:3
                for half in range(2):
                    p_idx = 2 * g + half
                    o0 = 2 * p_idx
                    bs, be = blk_of[o0]
                    if o0 == bs:
                        ob = obp.tile([128, OBLK_MAX * L], BF16, tag="ob",
                                      name=f"ob{p_idx}")
                    sl = ob[:, (o0 - bs) * L:(o0 - bs + 2) * L]
                    if p_idx % 8 in DRAIN_ACT:
                        nc.scalar.copy(sl, packs[half][:])
                    else:
                        nc.vector.tensor_copy(sl, packs[half][:])
                    if o0 + 2 == be:
                        nc.sync.dma_start(
                            out_d[:, bs * L:be * L], ob[:, 0:(be - bs) * L])
                # build one aw block ahead (after drains: drain priority)
                if g + 3 < O // 4:
                    build_aw4(g + 3)

    nc.compile()
    return nc


_CACHE = {}


def _get_nc():
    if "nc" not in _CACHE:
        _CACHE["nc"] = _build()
    return _CACHE["nc"]


def _make_in_maps(x, ln_gamma, ln_beta, w1, b1, w2, b2, w3, b3, w4):
    x = np.ascontiguousarray(x, dtype=np.float32)
    g = np.asarray(ln_gamma, np.float32)
    be = np.asarray(ln_beta, np.float32)
    w1 = np.asarray(w1, np.float32)
    w2 = np.asarray(w2, np.float32)
    # fold the LN affine into the first-layer weights:
    # (xn*g + be) @ w = xn @ (g[:,None]*w) + be @ w
    w1g = g[:, None] * w1
    w2g = g[:, None] * w2
    b1e = np.asarray(b1, np.float32) + be @ w1
    b2e = np.asarray(b2, np.float32) + be @ w2
    w3c = np.asarray(w3, np.float32)
    w4f = np.asarray(w4, np.float32)
    b3f = np.asarray(b3, np.float32)
    w24 = w2g @ w4f                      # (D, O)
    nbias = b3f - b2e @ w4f              # (O,)

    bf = ml_dtypes.bfloat16
    wbfa = np.zeros((128, WA_N), dtype=bf)
    wbfa[:, WA_ID:WA_ID + 128] = np.eye(128, dtype=np.float32).astype(bf)
    wbfa[:, WA_W1:WA_W1 + 256] = \
        w1g.reshape(2, 128, C).transpose(1, 0, 2).reshape(128, 256).astype(bf)

    wbfb = np.zeros((128, WB_N), dtype=bf)
    wbfb[:, WB_W2:WB_W2 + 256] = \
        w2g.reshape(2, 128, C).transpose(1, 0, 2).reshape(128, 256).astype(bf)
    wbfb[:, WB_W24:WB_W24 + 128] = \
        w24.reshape(2, 128, O).transpose(1, 0, 2).reshape(128, 128).astype(bf)
    wbfb[:, WB_W4:WB_W4 + O] = w4f.astype(bf)

    wf32 = np.zeros((128, WF_N), dtype=np.float32)
    wf32[:, WF_W3:WF_W3 + O] = w3c
    wf32[:, WF_B1] = b1e
    wf32[:, WF_B2] = b2e
    wf32[0:O, WF_NB] = nbias

    ones = np.ones((4, ONES_N), dtype=bf)

    in_maps = []
    for k in range(NCORES):
        bi, q = k // (NCORES // B), k % (NCORES // B)
        xb = x[:, bi, :]                                    # (L, D)
        xtiles = xb.reshape(NT, 128, D)                     # (NT, 128, D)
        # rotate: slot s holds input tile (q+s)%NT; slot 0 = own l-block
        order = [(q + s) % NT for s in range(NT)]
        xall = np.ascontiguousarray(
            xtiles[order].transpose(1, 0, 2).astype(bf))    # (128, NT, D)
        in_maps.append({"xall": xall, "wbfa": wbfa, "wbfb": wbfb,
                        "wf32": wf32, "ones": ones})
    return in_maps


def kernel_run(inputs, trace=False):
    nc = _get_nc()
    in_maps = _make_in_maps(**inputs)
    res = run_bass_kernel_spmd(
        nc, in_maps, core_ids=list(range(NCORES)), trace=trace,
    )
    out = np.empty((B, L, L, O), dtype=np.float32)
    for k in range(NCORES):
        bi, q = k // (NCORES // B), k % (NCORES // B)
        blk = np.asarray(res.results[k]["out"]).astype(np.float32)
        tmp = blk.reshape(LBLK, O, NT, 128)      # (l, o, slot, j)
        rows = slice(q * LBLK, (q + 1) * LBLK)
        for s in range(NT):
            t = (q + s) % NT
            out[bi, rows, t * 128:(t + 1) * 128, :] = \
                tmp[:, :, s, :].transpose(0, 2, 1)
    return out, res


def kernel(**inputs) -> np.ndarray:
    out, _ = kernel_run(inputs, trace=False)
    return out
